# revision 14
# baseline (speedup 1.0000x reference)
"""Grouped-query attention with sliding-window mask on 8 Trainium2 cores.

Sharding: core c handles (batch b = c // 4, kv-head group hk = c % 4).
Each core projects q (4 query heads) / k / v for its group, applies RoPE,
runs windowed attention, and computes a partial output projection
out_partial = attn_heads @ Wo[hk block].  The host sums the 4 partials
per batch.

All matmul operands are fp16 (same 1 cycle/row PE rate as fp32r fast
mode, half the HBM traffic; PSUM accumulation stays fp32).  Attention
runs in a scores-transposed layout: S^T[kj, (g,qi)] so all 4 heads share
one N=512 moving operand per key tile.  Softmax is max-free (scores are
bounded ~|5|): the denominator for all partitions at once comes from a
ones[128,128]-stationary matmul over the Pool-engine-accumulated sum of
the exp blocks (one 512-wide matmul per q tile), the reciprocal is a DVE
approx on the replicated [128, 512] PSUM bank, and normalization is a
single DVE multiply into fp16 SBUF -- no partition-broadcast matmul.

DMA issue cost is ~0.6us per dma_start on the issuing engine's DGE, so
transfers are batched into few large descriptors and spread across
engine queues: x rides SP+DVE (8-chunk halves per quarter), weights ride
the Pool queue (quiet after startup), output stores ride the SP queue
(quiet after phase A).

Scheduling: engines execute their instruction streams in order, so the
issue order is staged to keep the PE busy: in phase A the PE transposes
of tile T are issued behind the projections of tile T+1 (RoPE for T runs
in that window); in phase B the output projection of q-tile qt-2 is
issued between the scores and the softmax-consumers of qt, hiding the
exp/denominator/reciprocal chain.  Problem constants are hardcoded (B=2,
N=2048, E=2048, H=16, G=4, D=128, WIN=256).
"""

import sys

for _p in ("/opt/trn_rl_repo", "/opt/pypackages"):
    if _p not in sys.path:
        sys.path.insert(0, _p)

from contextlib import ExitStack

import numpy as np

import concourse.bacc as bacc
import concourse.bass as bass
import concourse.mybir as mybir
import concourse.tile as tile
from concourse.bass_utils import run_bass_kernel_spmd

B, N, E = 2, 2048, 2048
H, G, WIN = 16, 4, 256
HK = H // G          # 4 kv heads
D = E // H           # 128
SCALE = D ** -0.5
NCORES = 8
P = 128
NT = N // P          # 16 n-tiles
EC = E // P          # 16 contraction chunks
QTR = N // 4         # xT streamed in quarter-columns
F32 = mybir.dt.float32
F16 = mybir.dt.float16
MASK_VAL = -30000.0  # fits fp16; exp() underflows to 0

_compiled = {}


def _rope(nc, rtmp, dst, src, c_ap, s_ap, nblocks):
    """RoPE on [128, nblocks*128] (pairs adjacent along free dim), all
    head-blocks in one strided op set.

    dst[2t]   = src[2t]*cos[t] - src[2t+1]*sin[t]
    dst[2t+1] = src[2t]*sin[t] + src[2t+1]*cos[t]
    """
    sb = src[:]
    db = dst[:]
    hd = D // 2
    x0 = bass.AP(sb.tensor, sb.offset + 0, [sb.ap[0], [P, nblocks], [2, hd]])
    x1 = bass.AP(sb.tensor, sb.offset + 1, [sb.ap[0], [P, nblocks], [2, hd]])
    d0 = bass.AP(db.tensor, db.offset + 0, [db.ap[0], [P, nblocks], [2, hd]])
    d1 = bass.AP(db.tensor, db.offset + 1, [db.ap[0], [P, nblocks], [2, hd]])
    cb = c_ap
    c3 = bass.AP(cb.tensor, cb.offset, [cb.ap[0], [0, nblocks], cb.ap[1]])
    s3 = bass.AP(s_ap.tensor, s_ap.offset, [s_ap.ap[0], [0, nblocks], s_ap.ap[1]])
    t0 = rtmp.tile([P, nblocks * hd], F32, tag="t0", name="t0")
    t1 = rtmp.tile([P, nblocks * hd], F32, tag="t1", name="t1")
    t0v = t0[:].rearrange("p (b d) -> p b d", d=hd)
    t1v = t1[:].rearrange("p (b d) -> p b d", d=hd)
    nc.vector.tensor_mul(t0v, x0, c3)
    nc.vector.tensor_mul(t1v, x1, s3)
    nc.vector.tensor_sub(d0, t0v, t1v)
    t2 = rtmp.tile([P, nblocks * hd], F32, tag="t2", name="t2")
    t3 = rtmp.tile([P, nblocks * hd], F32, tag="t3", name="t3")
    t2v = t2[:].rearrange("p (b d) -> p b d", d=hd)
    t3v = t3[:].rearrange("p (b d) -> p b d", d=hd)
    nc.vector.tensor_mul(t2v, x0, s3)
    nc.vector.tensor_mul(t3v, x1, c3)
    nc.vector.tensor_add(d1, t2v, t3v)


def _bcast_g(ap):
    """[128, 128] AP -> [128, G, 128] with a 0-step head dim."""
    return bass.AP(ap.tensor, ap.offset, [ap.ap[0], [0, G], ap.ap[1]])


def _build():
    nc = bacc.Bacc("TRN2", target_bir_lowering=False, debug=False)

    xt_d = nc.dram_tensor("xt", [E, N], F16, kind="ExternalInput")
    wq_d = nc.dram_tensor("wq", [E, G * D], F16, kind="ExternalInput")
    wkv_d = nc.dram_tensor("wkv", [E, 2 * D], F16, kind="ExternalInput")
    wo_d = nc.dram_tensor("wo", [G * D, E], F16, kind="ExternalInput")
    cos_d = nc.dram_tensor("coss", [N, D // 2], F16, kind="ExternalInput")
    sin_d = nc.dram_tensor("sins", [N, D // 2], F16, kind="ExternalInput")
    maskt_d = nc.dram_tensor("maskt", [P, 2 * P], F32, kind="ExternalInput")
    eye_d = nc.dram_tensor("eye", [P, P], F16, kind="ExternalInput")
    onsq_d = nc.dram_tensor("onsq", [P, P], F16, kind="ExternalInput")
    out_d = nc.dram_tensor("out", [N, E], F16, kind="ExternalOutput")

    xt3 = xt_d.ap().rearrange("(c p) n -> p c n", p=P)
    wq3 = wq_d.ap().rearrange("(c p) m -> p c m", p=P)
    wkv3 = wkv_d.ap().rearrange("(c p) m -> p c m", p=P)
    wo3 = wo_d.ap().rearrange("(g p) e -> g p e", p=P)

    with tile.TileContext(nc) as tc, ExitStack() as top:
        pers = top.enter_context(tc.tile_pool(name="pers", bufs=1))
        # qT layout: [d, (qt, g, qi)] -> col = qt*512 + g*128 + qi
        qt_sb = pers.tile([P, G * N], F16, tag="qt")
        kt_sb = pers.tile([P, N], F16, tag="kt")          # [d, n]
        v_sb = pers.tile([P, N], F16, tag="v")            # blk t: v[t*128+p, d]
        cos_sb = pers.tile([P, NT * (D // 2)], F16, tag="cos")
        sin_sb = pers.tile([P, NT * (D // 2)], F16, tag="sin")
        maskt_sb = pers.tile([P, 2 * P], F32, tag="maskt")
        eye_sb = pers.tile([P, P], F16, tag="eye")
        ones_sq = pers.tile([P, P], F16, tag="ones_sq")   # denominator lhsT
        wq_sb = pers.tile([P, EC * G * D], F16, tag="wq")   # chunk e at e*512
        wkv_sb = pers.tile([P, EC * 2 * D], F16, tag="wkv")  # chunk e at e*256

        wo_pool = top.enter_context(tc.tile_pool(name="wo", bufs=G))
        wos = [wo_pool.tile([P, E], F16, tag="wo", name="wot") for _ in range(G)]

        wq4 = wq_sb[:].rearrange("p (c m) -> p c m", m=G * D)
        wkv4 = wkv_sb[:].rearrange("p (c m) -> p c m", m=2 * D)

        # ---------------- Phase A: projections + RoPE + transposes ---------
        with ExitStack() as pha:
            xt_pool = pha.enter_context(tc.tile_pool(name="xtp", bufs=2))
            qrot_pool = pha.enter_context(tc.tile_pool(name="qrot", bufs=2))
            krot_pool = pha.enter_context(tc.tile_pool(name="krot", bufs=2))
            rtmp = pha.enter_context(tc.tile_pool(name="rtmp", bufs=4))
            qps_pool = pha.enter_context(
                tc.tile_pool(name="qps", bufs=3, space="PSUM"))
            kvps_pool = pha.enter_context(
                tc.tile_pool(name="kvps", bufs=3, space="PSUM"))
            trps_pool = pha.enter_context(
                tc.tile_pool(name="trps", bufs=2, space="PSUM"))

            pend = []

            def flush_transposes():
                for kind, rot, T in pend:
                    if kind == "q":
                        for g in range(G):
                            tq = trps_pool.tile([P, P], F16, tag="trq",
                                                name="trq")
                            nc.tensor.transpose(
                                tq[:], rot[:, g * P:(g + 1) * P], eye_sb[:])
                            nc.scalar.copy(
                                qt_sb[:, T * 4 * P + g * P:
                                      T * 4 * P + (g + 1) * P],
                                tq[:])
                    else:
                        tk = trps_pool.tile([P, P], F16, tag="trq", name="trk")
                        nc.tensor.transpose(tk[:], rot[:], eye_sb[:])
                        nc.scalar.copy(kt_sb[:, T * P:(T + 1) * P], tk[:])
                pend.clear()

            def seg_views(bufs, widths):
                views, lo = [], 0
                for b, w in zip(bufs, widths):
                    views.append((b[:].rearrange("p (c n) -> p c n", n=QTR),
                                  lo, w))
                    lo += w
                return views

            # Quarter 0 x: 4-chunk segments on the SP DGE queue (low first-
            # segment latency); quarters 1-3 are prefetched from inside the
            # previous quarter's tile loop on the Act queue, so their
            # transfers don't steal startup HBM bandwidth from the weights.
            q0bufs = [xt_pool.tile([P, 4 * QTR], F16, tag=f"xs{s}", name="xs")
                      for s in range(4)]
            xsegs = seg_views(q0bufs, [4, 4, 4, 4])
            for (v4, lo, w) in xsegs:
                nc.sync.dma_start(v4, xt3[:, lo:lo + w, 0:QTR])

            # wq rides the Act DGE queue (idle until phase B); wkv + consts
            # ride the Pool queue.  Wo is deferred to quarter 2 -- it is not
            # needed until phase B and would steal startup bandwidth.
            for c0 in range(0, EC, 4):
                nc.scalar.dma_start(wq4[:, c0:c0 + 4, :], wq3[:, c0:c0 + 4, :])
            for c0 in range(0, EC, 4):
                nc.gpsimd.dma_start(
                    wkv4[:, c0:c0 + 4, :], wkv3[:, c0:c0 + 4, :])
            nc.gpsimd.dma_start(
                cos_sb[:].rearrange("p (t d) -> p t d", d=D // 2),
                cos_d.ap().rearrange("(t p) d -> p t d", p=P))
            nc.gpsimd.dma_start(
                sin_sb[:].rearrange("p (t d) -> p t d", d=D // 2),
                sin_d.ap().rearrange("(t p) d -> p t d", p=P))
            nc.gpsimd.dma_start(eye_sb[:], eye_d.ap())
            nc.gpsimd.dma_start(maskt_sb[:], maskt_d.ap())
            nc.gpsimd.dma_start(ones_sq[:], onsq_d.ap())

            for qtr in range(4):
                def xchunk(e, tt, segs=xsegs):
                    for (v4, lo, w) in segs:
                        if lo <= e < lo + w:
                            return v4[:, e - lo, tt * P:(tt + 1) * P]

                for t in range(4):
                    T = qtr * 4 + t
                    q_ps = qps_pool.tile([P, G * D], F32, tag="qps")
                    kv_ps = kvps_pool.tile([P, 2 * D], F32, tag="kvps")
                    for e in range(EC):
                        nc.tensor.matmul(
                            q_ps[:], xchunk(e, t), wq4[:, e, :],
                            start=(e == 0), stop=(e == EC - 1))
                    # PE transposes for the previous tile; their RoPE inputs
                    # finished during the projections above.
                    flush_transposes()
                    for e in range(EC):
                        nc.tensor.matmul(
                            kv_ps[:], xchunk(e, t), wkv4[:, e, :],
                            start=(e == 0), stop=(e == EC - 1))

                    c_ap = cos_sb[:, T * (D // 2):(T + 1) * (D // 2)]
                    s_ap = sin_sb[:, T * (D // 2):(T + 1) * (D // 2)]
                    q_rot = qrot_pool.tile([P, G * D], F16, tag="qrot")
                    k_rot = krot_pool.tile([P, D], F16, tag="krot")
                    _rope(nc, rtmp, q_rot, q_ps, c_ap, s_ap, G)
                    _rope(nc, rtmp, k_rot, kv_ps, c_ap, s_ap, 1)
                    nc.scalar.copy(v_sb[:, T * P:(T + 1) * P], kv_ps[:, D:2 * D])
                    pend.append(("q", q_rot, T))
                    pend.append(("k", k_rot, T))
                    if t == 0 and qtr < 3:
                        # next quarter's x halves on the SP queue (idle now)
                        nsl = slice((qtr + 1) * QTR, (qtr + 2) * QTR)
                        nh = []
                        for h in range(2):
                            xh = xt_pool.tile([P, 8 * QTR], F16,
                                              tag=f"xh{h}", name="xh")
                            xh4 = xh[:].rearrange("p (c n) -> p c n", n=QTR)
                            nc.sync.dma_start(
                                xh4, xt3[:, 8 * h:8 * h + 8, nsl])
                            nh.append((xh4, 8 * h, 8))
                        xsegs = nh
                    if t == 0 and qtr == 2:
                        for g in range(G):
                            nc.gpsimd.dma_start(wos[g][:], wo3[g])
            flush_transposes()

        # ---------------- Phase B: attention + output projection -----------
        with ExitStack() as phb:
            ex_pool = phb.enter_context(tc.tile_pool(name="ex", bufs=2))
            smm_pool = phb.enter_context(tc.tile_pool(name="smm", bufs=2))
            esum_pool = phb.enter_context(tc.tile_pool(name="esum", bufs=4))
            rec_pool = phb.enter_context(tc.tile_pool(name="rec", bufs=2))
            ao_pool = phb.enter_context(tc.tile_pool(name="aosb", bufs=3))
            osb_pool = phb.enter_context(tc.tile_pool(name="osb", bufs=2))
            sps_pool = phb.enter_context(
                tc.tile_pool(name="sps", bufs=3, space="PSUM"))
            denps_pool = phb.enter_context(
                tc.tile_pool(name="denps", bufs=1, space="PSUM"))
            aops_pool = phb.enter_context(
                tc.tile_pool(name="aops", bufs=2, space="PSUM"))
            wops_pool = phb.enter_context(
                tc.tile_pool(name="wops", bufs=2, space="PSUM"))

            W = G * P  # 512: (g, qi) moving width

            def outproj(ao_sb, qt):
                out_sb = osb_pool.tile([P, E], F16, tag="outsb")
                for eb in range(4):
                    wo_ps = wops_pool.tile([P, 512], F32, tag="wops")
                    for g in range(G):
                        nc.tensor.matmul(
                            wo_ps[:],
                            ao_sb[:, g * P:(g + 1) * P],
                            wos[g][:, eb * 512:(eb + 1) * 512],
                            start=(g == 0), stop=(g == G - 1))
                    dst = out_sb[:, eb * 512:(eb + 1) * 512]
                    # split PSUM->SBUF drains across Act and DVE (Pool
                    # cannot read PSUM)
                    if eb % 2 == 0:
                        nc.scalar.copy(dst, wo_ps[:])
                    else:
                        nc.vector.tensor_copy(dst, wo_ps[:])
                    # output stores ride the SP DGE queue (quiet in phase B)
                    nc.sync.dma_start(
                        out_d.ap()[qt * P:(qt + 1) * P,
                                   eb * 512:(eb + 1) * 512], dst)

            prevs = []
            for qt in range(NT):
                nk = min(qt, 2) + 1
                kb0 = qt - (nk - 1)
                exps = ex_pool.tile([P, 3 * W], F16, tag="exps")
                # -- scores (PE back-to-back), mask+exp chase them --
                for j in range(nk):
                    kb = kb0 + j
                    dabs = kb - qt          # -2, -1, or 0
                    st_ps = sps_pool.tile([P, W], F32, tag="stps")
                    nc.tensor.matmul(
                        st_ps[:],
                        kt_sb[:, kb * P:(kb + 1) * P],
                        qt_sb[:, qt * W:(qt + 1) * W],
                        start=True, stop=True)
                    eblk = exps[:, j * W:(j + 1) * W]
                    if dabs == -1:
                        nc.scalar.activation(
                            eblk, st_ps[:], mybir.ActivationFunctionType.Exp)
                    else:
                        mblk = maskt_sb[:, 0:P] if dabs == -2 \
                            else maskt_sb[:, P:2 * P]
                        st_sb = smm_pool.tile([P, W], F16, tag="stsb")
                        nc.vector.tensor_add(
                            st_sb[:].rearrange("p (g q) -> p g q", g=G),
                            st_ps[:].rearrange("p (g q) -> p g q", g=G),
                            _bcast_g(mblk))
                        nc.scalar.activation(
                            eblk, st_sb[:], mybir.ActivationFunctionType.Exp)

                # -- output projection of the qt-2 tile fills the PE while
                #    the exp/denominator chain of qt lands --
                if len(prevs) == 2:
                    outproj(*prevs.pop(0))

                # -- AV first (needs only exps), then the denominator matmul
                #    over the Pool-accumulated exp sum (it may trail without
                #    blocking AV in the PE queue) --
                if nk == 1:
                    den_src = exps[:, 0:W]
                elif nk == 2:
                    es = esum_pool.tile([P, W], F16, tag="esum")
                    nc.gpsimd.tensor_add(es[:], exps[:, 0:W], exps[:, W:2 * W])
                    den_src = es[:]
                else:
                    esa = esum_pool.tile([P, W], F16, tag="esum")
                    nc.gpsimd.tensor_add(esa[:], exps[:, 0:W], exps[:, W:2 * W])
                    es = esum_pool.tile([P, W], F16, tag="esum")
                    nc.gpsimd.tensor_add(es[:], esa[:], exps[:, 2 * W:3 * W])
                    den_src = es[:]
                ao_ps = aops_pool.tile([P, W], F32, tag="aops")
                for j in range(nk):
                    kb = kb0 + j
                    nc.tensor.matmul(
                        ao_ps[:],
                        v_sb[:, kb * P:(kb + 1) * P],
                        exps[:, j * W:(j + 1) * W],
                        start=(j == 0), stop=(j == nk - 1))
                den_ps = denps_pool.tile([P, W], F32, tag="denps")
                nc.tensor.matmul(
                    den_ps[:], ones_sq[:], den_src, start=True, stop=True)
                rec = rec_pool.tile([P, W], F32, tag="rec")
                nc.vector.reciprocal_approx_fast(rec[:], den_ps[:])
                ao_sb = ao_pool.tile([P, W], F16, tag="aosb")
                nc.vector.tensor_mul(ao_sb[:], ao_ps[:], rec[:])
                prevs.append((ao_sb, qt))
            for pv in prevs:
                outproj(*pv)

    nc.compile()
    return nc


def _host_inputs(x, rope_cos, rope_sin, Wq, Wk, Wv, Wo):
    """Build the 8 per-core input maps."""
    band = np.full((P, 3 * P), MASK_VAL, dtype=np.float32)
    r = np.arange(P)[:, None]
    c = np.arange(3 * P)[None, :]
    band[(c > r) & (c <= r + WIN)] = 0.0
    # transposed mask blocks: [:, :128] for key-tile offset -2,
    # [:, 128:] (causal) for offset 0
    maskt = np.ascontiguousarray(np.concatenate(
        [band[:, 0:P].T, band[:, 2 * P:3 * P].T], axis=1))
    eye = np.eye(P, dtype=np.float16)
    onsq = np.ones((P, P), dtype=np.float16)

    in_maps = []
    for core in range(NCORES):
        b, hk = divmod(core, HK)
        xt = np.ascontiguousarray(x[b].T.astype(np.float16))
        wq = np.ascontiguousarray(
            (Wq[:, hk * G * D:(hk + 1) * G * D] * SCALE).astype(np.float16))
        wkv = np.ascontiguousarray(np.concatenate(
            [Wk[:, hk * D:(hk + 1) * D], Wv[:, hk * D:(hk + 1) * D]],
            axis=1).astype(np.float16))
        wo = np.ascontiguousarray(
            Wo[hk * G * D:(hk + 1) * G * D, :].astype(np.float16))
        in_maps.append({
            "xt": xt,
            "wq": wq,
            "wkv": wkv,
            "wo": wo,
            "coss": np.ascontiguousarray(rope_cos[b].astype(np.float16)),
            "sins": np.ascontiguousarray(rope_sin[b].astype(np.float16)),
            "maskt": maskt,
            "eye": eye,
            "onsq": onsq,
        })
    return in_maps


def _run(inputs, trace=False, **kw):
    if "nc" not in _compiled:
        _compiled["nc"] = _build()
    nc = _compiled["nc"]
    in_maps = _host_inputs(**inputs)
    res = run_bass_kernel_spmd(nc, in_maps, list(range(NCORES)), trace=trace, **kw)
    out = np.zeros((B, N, E), dtype=np.float32)
    for core in range(NCORES):
        b = core // HK
        out[b] += res.results[core]["out"].astype(np.float32)
    return out, res


def kernel(**inputs):
    out, _ = _run(inputs, trace=False)
    return out
